# revision 1
# baseline (speedup 1.0000x reference)
"""AttentionPairBias kernel for 8 Trainium2 NeuronCores.

Sharding: rows of the query sequence (S=1024) are split across the 8 cores
(128 rows each). The pair tensor z's bias contribution, the softmax and the
output rows are all embarrassingly parallel in the query dimension, so no
collectives are needed; each core reads its own 128x1024x128 slice of z.

Per-core pipeline:
  1. XBAR DMA-transpose loads z rows as zT [c=128, t] (bf16).
  2. ACT squares zT; one PE matmul with an augmented weight matrix
     [ln_w*Wz^T | ones/128 | 0] plus an accumulating second matmul on zT^2
     produces y[h,t] (pair-bias pre-LN), mean and E[z^2] rows in PSUM.
     Four query rows are packed into each PSUM bank via col-tiling.
  3. y/mu/E[z^2] round-trip through DRAM to re-slice [h,t]-per-row into
     [row, t]-per-head tiles.
  4. r = rsqrt(var+eps) via Ln/Exp; bias_h = r*(y_h - c1[h]*mu) (the ln_b
     term is dropped: it is constant along t, softmax-invariant).
  5. Per head: scores = qk/sqrt(hd) + bias -> PE transpose -> exp on ACT
     (max-subtraction-free: |scores| < 4) -> A@[V|1] gives o and the softmax
     denominator in one accumulation chain.
  6. sigmoid gate, output projection.
"""

import os
import sys
import types
import numpy as np

for _p in ("/opt/trn_rl_repo", "/root/.axon_site/_ro/trn_rl_repo"):
    if os.path.isdir(_p) and _p not in sys.path:
        sys.path.append(_p)

import ml_dtypes
from contextlib import ExitStack

import concourse.bass as bass
import concourse.mybir as mybir
import concourse.tile as tile
from concourse import bacc
from concourse.bass import ds, ts
from concourse.masks import make_identity

BF16 = mybir.dt.bfloat16
FP32 = mybir.dt.float32
AF = mybir.ActivationFunctionType
ALU = mybir.AluOpType

S = 1024
D = 768
H = 16
HD = 48
HDP = 64            # padded head dim (2 heads per 128-partition block)
DP = H * HDP        # 1024
DZ = 128
EPS = 1e-5
N_CORES = 8
RPC = S // N_CORES  # 128 rows per core
ISQ = float(HD) ** -0.5

_CACHE = {}


def _build(c1):
    """Build the per-core SPMD program. c1[h] = sum_c ln_w[c]*Wz[h,c] are
    baked as immediates."""
    nc = bacc.Bacc("TRN2", target_bir_lowering=False, debug=False,
                   num_devices=N_CORES)

    zb = nc.dram_tensor("zb", [RPC, S, DZ], BF16, kind="ExternalInput").ap()
    sT = nc.dram_tensor("sT", [D, S], BF16, kind="ExternalInput").ap()
    sTc = nc.dram_tensor("sTc", [D, RPC], BF16, kind="ExternalInput").ap()
    WqT = nc.dram_tensor("WqT", [D, DP], BF16, kind="ExternalInput").ap()
    WkT = nc.dram_tensor("WkT", [D, DP], BF16, kind="ExternalInput").ap()
    WvT = nc.dram_tensor("WvT", [D, DP], BF16, kind="ExternalInput").ap()
    WgT = nc.dram_tensor("WgT", [D, D], BF16, kind="ExternalInput").ap()
    WoT = nc.dram_tensor("WoT", [D, D], BF16, kind="ExternalInput").ap()
    bqs = nc.dram_tensor("bqs", [DP], FP32, kind="ExternalInput").ap()
    W18 = nc.dram_tensor("W18", [DZ, 32], BF16, kind="ExternalInput").ap()
    Wss = nc.dram_tensor("Wss", [DZ, 32], BF16, kind="ExternalInput").ap()
    out = nc.dram_tensor("out", [RPC, D], FP32, kind="ExternalOutput").ap()

    with tile.TileContext(nc) as tc, ExitStack() as ctx:
        consts = ctx.enter_context(tc.tile_pool(name="consts", bufs=1))
        dram = ctx.enter_context(tc.tile_pool(name="dram", bufs=1, space="DRAM"))

        sT_sb = consts.tile([128, 6, S], BF16, name="sT_sb")
        nc.sync.dma_start(sT_sb[:], sT.rearrange("(a p) n -> p a n", p=128))
        sTc_sb = consts.tile([128, 6, RPC], BF16, name="sTc_sb")
        nc.scalar.dma_start(sTc_sb[:], sTc.rearrange("(a p) n -> p a n", p=128))
        wq_sb = consts.tile([128, 6, DP], BF16, name="wq_sb")
        nc.scalar.dma_start(wq_sb[:], WqT.rearrange("(a p) n -> p a n", p=128))
        wk_sb = consts.tile([128, 6, DP], BF16, name="wk_sb")
        nc.sync.dma_start(wk_sb[:], WkT.rearrange("(a p) n -> p a n", p=128))
        wv_sb = consts.tile([128, 6, DP], BF16, name="wv_sb")
        nc.sync.dma_start(wv_sb[:], WvT.rearrange("(a p) n -> p a n", p=128))
        wg_sb = consts.tile([128, 6, D], BF16, name="wg_sb")
        nc.scalar.dma_start(wg_sb[:], WgT.rearrange("(a p) n -> p a n", p=128))
        wo_sb = consts.tile([128, 6, D], BF16, name="wo_sb")
        nc.scalar.dma_start(wo_sb[:], WoT.rearrange("(a p) n -> p a n", p=128))
        w18_sb = consts.tile([128, 32], BF16, name="w18_sb")
        nc.sync.dma_start(w18_sb[:], W18[:])
        wss_sb = consts.tile([128, 32], BF16, name="wss_sb")
        nc.sync.dma_start(wss_sb[:], Wss[:])
        bq_sb = consts.tile([128, 8], FP32, name="bq_sb")
        nc.sync.dma_start(bq_sb[:], bqs.rearrange("(b p) -> p b", p=128))
        ident = consts.tile([128, 128], BF16, name="ident")
        make_identity(nc, ident[:])
        eps_sb = consts.tile([128, 1], FP32, name="eps_sb")
        nc.vector.memset(eps_sb[:], EPS)

        kT_sb = consts.tile([128, 8, S], BF16, name="kT_sb")
        v_sb = consts.tile([128, 8, H, HDP + 1], BF16, name="v_sb")
        qT_sb = consts.tile([128, 8, RPC], BF16, name="qT_sb")
        g_sb = consts.tile([128, D], BF16, name="g_sb")
        oall = consts.tile([128, D], BF16, name="oall")
        mu_sb = consts.tile([128, S], BF16, name="mu_sb")
        ez2_sb = consts.tile([128, S], BF16, name="ez2_sb")
        r_sb = consts.tile([128, S], BF16, name="r_sb")
        u_sb = consts.tile([128, S], BF16, name="u_sb")
        var_sb = consts.tile([128, S], FP32, name="var_sb")

        y_dram = dram.tile([RPC, 18, S], BF16)

        nc.vector.memset(v_sb[:, :, :, HDP:HDP + 1], 1.0)

        # ---- stage B (projections) + stage C (pair-bias) share pools so
        # the scheduler can overlap z transposes with projection matmuls ----
        with tc.tile_pool(name="psA", bufs=2, space="PSUM") as psA, \
             tc.tile_pool(name="psY", bufs=3, space="PSUM") as psY, \
             tc.tile_pool(name="zwork", bufs=3) as zw, \
             tc.tile_pool(name="ypool", bufs=4) as yp:
            # kT (padded to HDP per head): [dout_block, t]
            for blk in range(8):
                for ch in range(2):
                    p = psA.tile([128, 512], FP32, tag="pA")
                    for ko in range(6):
                        nc.tensor.matmul(p[:], lhsT=wk_sb[:, ko, ts(blk, 128)],
                                         rhs=sT_sb[:, ko, ts(ch, 512)],
                                         start=(ko == 0), stop=(ko == 5))
                    nc.vector.tensor_copy(kT_sb[:, blk, ts(ch, 512)], p[:])
            # v (padded): [t_block, dout]
            for tb in range(8):
                for ch in range(2):
                    p = psA.tile([128, 512], FP32, tag="pA")
                    for ko in range(6):
                        nc.tensor.matmul(p[:], lhsT=sT_sb[:, ko, ts(tb, 128)],
                                         rhs=wv_sb[:, ko, ts(ch, 512)],
                                         start=(ko == 0), stop=(ko == 5))
                    nc.vector.tensor_copy(
                        v_sb[:, tb, ds(8 * ch, 8), 0:HDP],
                        p.rearrange("p (a b) -> p a b", a=8))
            # qT for own rows, scaled by 1/sqrt(hd), bias added
            for blk in range(8):
                p = psA.tile([128, 512], FP32, tag="pA", name="pQ")[:, :RPC]
                for ko in range(6):
                    nc.tensor.matmul(p[:], lhsT=wq_sb[:, ko, ts(blk, 128)],
                                     rhs=sTc_sb[:, ko, :],
                                     start=(ko == 0), stop=(ko == 5))
                nc.scalar.activation(qT_sb[:, blk, :], p[:], AF.Identity,
                                     bias=bq_sb[:, blk:blk + 1], scale=ISQ)
            # g for own rows
            for ch, w in ((0, 512), (1, 256)):
                p = psA.tile([128, 512], FP32, tag="pA")
                for ko in range(6):
                    nc.tensor.matmul(p[:, :w], lhsT=sTc_sb[:, ko, :],
                                     rhs=wg_sb[:, ko, ds(512 * ch, w)],
                                     start=(ko == 0), stop=(ko == 5))
                nc.vector.tensor_copy(g_sb[:, ds(512 * ch, w)], p[:, :w])

            # ---- stage C: pair-bias pipeline over own z rows ----
            # One 4-row 1MB XBAR transpose per group (single ring: concurrent
            # xbar transposes corrupt). bf16 PSUM output [32, 1024] per row
            # packs the whole group into ONE bank via col-tiling; y then ss
            # accumulate per row, kept in issue order (has_written is
            # bank-wide on HW, per-element only in CoreSim).
            for grp in range(RPC // 4):
                ps = [psY.tile([128, 512], FP32, tag=f"py{ch}", name=f"py{ch}")
                      for ch in range(2)]
                zT4 = zw.tile([128, 4 * S], BF16, tag="zT4")
                nc.sync.dma_start_transpose(
                    zT4[:], zb[ds(4 * grp, 4)].rearrange("r t c -> (r t) c"))
                sqh = []
                for half in range(2):
                    sq2 = zw.tile([128, 2 * S], BF16, tag="sq2", name=f"sq2_{half}")
                    if half == 0:
                        nc.vector.tensor_tensor(sq2[:], zT4[:, 0:2 * S],
                                                zT4[:, 0:2 * S], ALU.mult)
                    else:
                        nc.scalar.activation(sq2[:], zT4[:, 2 * S:4 * S], AF.Square)
                    sqh.append(sq2)
                prev_ss = [None, None]
                for j in range(4):
                    for ch in range(2):
                        o32 = ps[ch][ds(32 * j, 32), :]
                        m1 = nc.tensor.matmul(
                            o32, lhsT=w18_sb[:],
                            rhs=zT4[:, ds(S * j + 512 * ch, 512)],
                            start=True, stop=False,
                            tile_position=(0, 32 * j))
                        if prev_ss[ch] is not None:
                            tile.add_dep_helper(m1.ins, prev_ss[ch].ins,
                                                sync=False,
                                                reason="bank has_written order")
                        prev_ss[ch] = nc.tensor.matmul(
                            o32, lhsT=wss_sb[:],
                            rhs=sqh[j // 2][:, ds(S * (j % 2) + 512 * ch, 512)],
                            start=False, stop=True,
                            tile_position=(0, 32 * j))
                y4 = yp.tile([128, 2, 512], BF16, tag="y4")
                nc.vector.tensor_copy(y4[:, 0, :], ps[0][:])
                nc.vector.tensor_copy(y4[:, 1, :], ps[1][:])
                # y writes on the otherwise-idle SWDGE ring
                eng = nc.gpsimd
                for j in range(4):
                    eng.dma_start(
                        y_dram[4 * grp + j].rearrange("s (a b) -> s a b", a=2),
                        y4[ds(32 * j, 18), :, :])

        # ---- stage D: r and u from round-tripped stats ----
        nc.sync.dma_start(mu_sb[:], y_dram[:, 16, :])
        nc.sync.dma_start(ez2_sb[:], y_dram[:, 17, :])
        nc.vector.tensor_tensor(var_sb[:], mu_sb[:], mu_sb[:], ALU.mult)
        nc.vector.tensor_tensor(var_sb[:], ez2_sb[:], var_sb[:], ALU.subtract)
        nc.scalar.activation(var_sb[:], var_sb[:], AF.Ln, bias=eps_sb[:])
        nc.scalar.activation(r_sb[:], var_sb[:], AF.Exp, scale=-0.5)
        nc.vector.tensor_tensor(u_sb[:], mu_sb[:], r_sb[:], ALU.mult)

        # ---- stage E: attention per head ----
        with tc.tile_pool(name="psE", bufs=2, space="PSUM") as psE, \
             tc.tile_pool(name="head", bufs=2) as hw_pool:
            for h in range(H):
                po2, blk = 64 * (h % 2), h // 2
                y_h = hw_pool.tile([128, S], BF16, tag="yh")
                if h % 2 == 0:
                    nc.sync.dma_start(y_h[:], y_dram[:, h, :])
                else:
                    nc.scalar.dma_start(y_h[:], y_dram[:, h, :])
                t1 = hw_pool.tile([128, S], BF16, tag="t1")
                nc.vector.tensor_tensor(t1[:], y_h[:], r_sb[:], ALU.mult)
                t2 = hw_pool.tile([128, S], BF16, tag="t2")
                nc.vector.tensor_scalar(t2[:], u_sb[:], -c1[h], None, op0=ALU.mult)
                sc = hw_pool.tile([128, S], BF16, tag="sc")
                for ch in range(2):
                    pq = psE.tile([128, 512], FP32, tag="qk")
                    nc.tensor.matmul(pq[:],
                                     lhsT=qT_sb[ds(po2, 64), blk, :],
                                     rhs=kT_sb[ds(po2, 64), blk, ts(ch, 512)],
                                     start=True, stop=True)
                    nc.vector.tensor_tensor(sc[:, ts(ch, 512)], pq[:],
                                            t1[:, ts(ch, 512)], ALU.add)
                nc.vector.tensor_tensor(sc[:], sc[:], t2[:], ALU.add)
                aT = hw_pool.tile([128, 8, 128], BF16, tag="aT")
                for half in range(2):
                    pt = psE.tile([128, 512], BF16, tag="pt")
                    for jj in range(4):
                        nc.tensor.transpose(pt[:, ts(jj, 128)],
                                            sc[:, ts(4 * half + jj, 128)],
                                            ident[:])
                    nc.scalar.activation(aT[:, ds(4 * half, 4), :],
                                         pt.rearrange("p (a b) -> p a b", a=4),
                                         AF.Exp)
                po = psE.tile([128, HDP + 1], FP32, tag="po")
                for tb in range(8):
                    nc.tensor.matmul(po[:], lhsT=aT[:, tb, :],
                                     rhs=v_sb[:, tb, h, :],
                                     start=(tb == 0), stop=(tb == 7))
                dr = hw_pool.tile([128, 1], FP32, tag="dr")
                nc.vector.reciprocal(dr[:], po[:, HDP:HDP + 1])
                nc.vector.tensor_scalar(oall[:, ds(HD * h, HD)], po[:, 0:HD],
                                        dr[:], None, op0=ALU.mult)

            # ---- stage F: gate + output projection ----
            sig = hw_pool.tile([128, D], BF16, tag="sig")
            nc.scalar.activation(sig[:], g_sb[:], AF.Sigmoid)
            og = hw_pool.tile([128, D], BF16, tag="og")
            nc.vector.tensor_tensor(og[:], oall[:], sig[:], ALU.mult)
            ogT = hw_pool.tile([128, 6, 128], BF16, tag="ogT")
            for half, n in ((0, 4), (1, 2)):
                pt = psE.tile([128, 512], BF16, tag="pt")
                for jj in range(n):
                    nc.tensor.transpose(pt[:, ts(jj, 128)],
                                        og[:, ts(4 * half + jj, 128)], ident[:])
                nc.vector.tensor_copy(
                    ogT[:, ds(4 * half, n), :],
                    pt.rearrange("p (a b) -> p a b", a=4)[:, 0:n, :])
            out_sb = hw_pool.tile([128, D], FP32, tag="outsb")
            for ch, w in ((0, 512), (1, 256)):
                pf = psE.tile([128, 512], FP32, tag="qk")
                for ko in range(6):
                    nc.tensor.matmul(pf[:, :w], lhsT=ogT[:, ko, :],
                                     rhs=wo_sb[:, ko, ds(512 * ch, w)],
                                     start=(ko == 0), stop=(ko == 5))
                nc.vector.tensor_copy(out_sb[:, ds(512 * ch, w)], pf[:, :w])
            nc.sync.dma_start(out[:], out_sb[:])

    nc.compile()
    return nc


def _prep(inputs):
    bf = ml_dtypes.bfloat16
    s = np.asarray(inputs["s"], np.float32)[0]
    z = np.asarray(inputs["z"], np.float32)[0]
    Wq = np.asarray(inputs["Wq"], np.float32)
    bq = np.asarray(inputs["bq"], np.float32)
    Wk = np.asarray(inputs["Wk"], np.float32)
    Wv = np.asarray(inputs["Wv"], np.float32)
    Wg = np.asarray(inputs["Wg"], np.float32)
    ln_w = np.asarray(inputs["ln_w"], np.float32)
    ln_b = np.asarray(inputs["ln_b"], np.float32)  # noqa: F841 (softmax-invariant)
    Wz = np.asarray(inputs["Wz"], np.float32)
    Wo = np.asarray(inputs["Wo"], np.float32)

    def pad_rows(W):
        Wp = np.zeros((DP, D), np.float32)
        for h in range(H):
            Wp[h * HDP:h * HDP + HD] = W[h * HD:(h + 1) * HD]
        return Wp

    zb = z.astype(bf)
    sT = np.ascontiguousarray(s.T).astype(bf)
    WqTp = np.ascontiguousarray(pad_rows(Wq).T).astype(bf)
    WkTp = np.ascontiguousarray(pad_rows(Wk).T).astype(bf)
    WvTp = np.ascontiguousarray(pad_rows(Wv).T).astype(bf)
    WgT = np.ascontiguousarray(Wg.T).astype(bf)
    WoT = np.ascontiguousarray(Wo.T).astype(bf)
    bq_p = np.zeros(DP, np.float32)
    for h in range(H):
        bq_p[h * HDP:h * HDP + HD] = bq[h * HD:(h + 1) * HD]
    bq_p *= ISQ

    Wzp = ln_w[None, :] * Wz                     # [H, DZ]
    W18 = np.zeros((DZ, 32), np.float32)
    W18[:, :H] = Wzp.T
    W18[:, 16] = 1.0 / DZ
    Wss = np.zeros((DZ, 32), np.float32)
    Wss[:, 17] = 1.0 / DZ
    c1 = [float(x) for x in Wzp.sum(-1)]

    shared = {
        "sT": sT, "WqT": WqTp, "WkT": WkTp, "WvT": WvTp, "WgT": WgT,
        "WoT": WoT, "bqs": bq_p, "W18": W18.astype(bf), "Wss": Wss.astype(bf),
    }
    in_maps = []
    for ci in range(N_CORES):
        rows = slice(ci * RPC, (ci + 1) * RPC)
        m = dict(shared)
        m["zb"] = np.ascontiguousarray(zb[rows])
        m["sTc"] = np.ascontiguousarray(sT[:, rows])
        in_maps.append(m)
    return in_maps, c1


def _install_ntff_hook():
    try:
        import antenv
        from trn_agent_boot.trn_boot import _ntff_profile_via_ctypes
        from concourse import bass_utils
        mod = types.ModuleType("antenv.axon_hooks")
        mod._hook = _ntff_profile_via_ctypes('/opt/axon/libaxon_pjrt.so')
        mod.set_axon_ntff_profile_hook = lambda h: setattr(mod, "_hook", h)
        mod.get_axon_ntff_profile_hook = lambda: mod._hook
        sys.modules["antenv.axon_hooks"] = mod
        antenv.axon_hooks = mod
        bass_utils.upload_artifacts = lambda tmpdir: tmpdir
    except Exception as e:  # profiling is best-effort
        print(f"ntff hook install failed: {e}", file=sys.stderr)


def run(inputs, trace=False):
    from concourse.bass_utils import run_bass_kernel_spmd
    in_maps, c1 = _prep(inputs)
    key = tuple(np.round(c1, 6))
    if key not in _CACHE:
        _CACHE[key] = _build(c1)
    nc = _CACHE[key]
    if trace:
        _install_ntff_hook()
    res = run_bass_kernel_spmd(nc, in_maps, core_ids=list(range(N_CORES)),
                               trace=trace)
    out = np.concatenate([res.results[i]["out"] for i in range(N_CORES)], axis=0)
    return out[None].astype(np.float32), res


def kernel(**inputs) -> np.ndarray:
    out, _ = run(inputs, trace=bool(os.environ.get("KERNEL_TRACE")))
    return out



# revision 26
# speedup vs baseline: 1.8270x; 1.8270x over previous
"""AttentionPairBias kernel for 8 Trainium2 NeuronCores.

Sharding: rows of the query sequence (S=1024) are split across the 8 cores
(128 rows each). The pair tensor z's bias contribution, the softmax and the
output rows are all embarrassingly parallel in the query dimension, so no
collectives are needed; each core reads its own 128x1024x128 slice of z.

Per-core pipeline (v2 — fp8 DoubleRow pair stream):
  1. z arrives host-transposed as [c=128, row, t] fp8(e4m3); plain DMA loads
     (no XBAR transpose).  Squares z^2 are computed on-device (DVE/ACT/Pool
     rotation) into the second DoubleRow slice.
  2. One fp8 DoubleRow matmul per (row, t-chunk) contracts c over both
     slices at once: slice0 weights = 64*(ln_w*Wz - c1/DZ | 1/DZ | 0),
     slice1 weights = 64*(0 | .. | 1/DZ) -> y'[h], 64*mu, 64*E[z^2] in one
     PSUM pass (the c1 mean-fold is baked into the weights; ln_b dropped:
     softmax-invariant).  4 rows pack one 2-bank PSUM tile via col-tiling.
  3. PSUM -> bf16 y4 copy (DVE/ACT alternate), one DMA per 4-row group to a
     flat DRAM scratch y_flat[32*row + m, t]; per-head tiles read back with
     a uniform-stride gather y_flat[h::32].
  4. var*4096 = 64*ez2_s - mu_s^2; r/64 = rsqrt-via-Ln/Exp(var4096+4096eps);
     bias_h = (r/64)*y'_s.
  5. Per head: scores = qk/sqrt(hd) + bias -> PE transpose -> exp on ACT
     (max-subtraction-free: |scores| < 4) -> A@[V|1] gives o and the softmax
     denominator in one accumulation chain.
  6. sigmoid gate, output projection.
"""

import os
import sys
import types
import numpy as np

for _p in ("/opt/trn_rl_repo", "/root/.axon_site/_ro/trn_rl_repo"):
    if os.path.isdir(_p) and _p not in sys.path:
        sys.path.append(_p)

import ml_dtypes
from contextlib import ExitStack

import concourse.bass as bass
import concourse.mybir as mybir
import concourse.tile as tile
from concourse import bacc
from concourse.bass import ds, ts
from concourse.masks import make_identity

F8 = mybir.dt.float8e4
BF16 = mybir.dt.bfloat16
FP32 = mybir.dt.float32
AF = mybir.ActivationFunctionType
ALU = mybir.AluOpType
DR = mybir.MatmulPerfMode.DoubleRow

S = 1024
D = 768
H = 16
HD = 48
HDP = 64            # padded head dim (2 heads per 128-partition block)
DP = H * HDP        # 1024
DZ = 128
EPS = 1e-5
N_CORES = 8
RPC = S // N_CORES  # 128 rows per core
ISQ = float(HD) ** -0.5
WS = 64.0           # fp8 weight scale
EPS_S = EPS * WS * WS

_CACHE = {}


def _build():
    nc = bacc.Bacc("TRN2", target_bir_lowering=False, debug=False,
                   num_devices=N_CORES)

    zb = nc.dram_tensor("zb", [DZ, RPC, S], F8, kind="ExternalInput").ap()
    w01 = nc.dram_tensor("w01", [DZ, 2, 32], F8, kind="ExternalInput").ap()
    sT = nc.dram_tensor("sT", [D, S], BF16, kind="ExternalInput").ap()
    sTc = nc.dram_tensor("sTc", [D, RPC], BF16, kind="ExternalInput").ap()
    WqT = nc.dram_tensor("WqT", [D, DP], BF16, kind="ExternalInput").ap()
    WkT = nc.dram_tensor("WkT", [D, DP], BF16, kind="ExternalInput").ap()
    WvT = nc.dram_tensor("WvT", [D, DP], BF16, kind="ExternalInput").ap()
    WgT = nc.dram_tensor("WgT", [D, D], BF16, kind="ExternalInput").ap()
    WoT = nc.dram_tensor("WoT", [D, D], BF16, kind="ExternalInput").ap()
    bqs = nc.dram_tensor("bqs", [DP], FP32, kind="ExternalInput").ap()
    out = nc.dram_tensor("out", [RPC, D], FP32, kind="ExternalOutput").ap()

    with tile.TileContext(nc) as tc, ExitStack() as ctx:
        consts = ctx.enter_context(tc.tile_pool(name="consts", bufs=1))
        dram = ctx.enter_context(tc.tile_pool(name="dram", bufs=1, space="DRAM"))

        sT_sb = consts.tile([128, 6, S], BF16, name="sT_sb")
        nc.sync.dma_start(sT_sb[:], sT.rearrange("(a p) n -> p a n", p=128))
        sTc_sb = consts.tile([128, 6, RPC], BF16, name="sTc_sb")
        nc.scalar.dma_start(sTc_sb[:], sTc.rearrange("(a p) n -> p a n", p=128))
        wq_sb = consts.tile([128, 6, DP], BF16, name="wq_sb")
        nc.scalar.dma_start(wq_sb[:], WqT.rearrange("(a p) n -> p a n", p=128))
        wk_sb = consts.tile([128, 6, DP], BF16, name="wk_sb")
        nc.sync.dma_start(wk_sb[:], WkT.rearrange("(a p) n -> p a n", p=128))
        wv_sb = consts.tile([128, 6, DP], BF16, name="wv_sb")
        nc.sync.dma_start(wv_sb[:], WvT.rearrange("(a p) n -> p a n", p=128))
        wg_sb = consts.tile([128, 6, D], BF16, name="wg_sb")
        nc.scalar.dma_start(wg_sb[:], WgT.rearrange("(a p) n -> p a n", p=128))
        wo_sb = consts.tile([128, 6, D], BF16, name="wo_sb")
        nc.scalar.dma_start(wo_sb[:], WoT.rearrange("(a p) n -> p a n", p=128))
        w01_sb = consts.tile([128, 2, 32], F8, name="w01_sb")
        nc.sync.dma_start(w01_sb[:], w01[:])
        bq_sb = consts.tile([128, 8], FP32, name="bq_sb")
        nc.sync.dma_start(bq_sb[:], bqs.rearrange("(b p) -> p b", p=128))
        ident = consts.tile([128, 128], BF16, name="ident")
        make_identity(nc, ident[:])
        eps_sb = consts.tile([128, 1], FP32, name="eps_sb")
        nc.vector.memset(eps_sb[:], EPS_S)

        kT_sb = consts.tile([128, 8, S], BF16, name="kT_sb")
        v_sb = consts.tile([128, 8, H, HDP + 1], BF16, name="v_sb")
        qT_sb = consts.tile([128, 8, RPC], BF16, name="qT_sb")
        g_sb = consts.tile([128, D], BF16, name="g_sb")
        oall = consts.tile([128, D], BF16, name="oall")
        mu_sb = consts.tile([128, S], BF16, name="mu_sb")
        ez2_sb = consts.tile([128, S], BF16, name="ez2_sb")
        r_sb = consts.tile([128, S], BF16, name="r_sb")
        var_sb = consts.tile([128, S], FP32, name="var_sb")

        y_flat = dram.tile([32 * RPC, S], BF16)

        nc.vector.memset(v_sb[:, :, :, HDP:HDP + 1], 1.0)

        # ---- stage B (projections) + stage C (pair-bias) share pools so
        # the scheduler can overlap z streaming with projection matmuls ----
        def square(eng, dst, src):
            if eng is nc.scalar:
                eng.activation(dst, src, AF.Square)
            else:
                eng.tensor_tensor(dst, src, src, ALU.mult)

        def copy(eng, dst, src):
            if eng is nc.scalar:
                eng.copy(dst, src)
            else:
                eng.tensor_copy(dst, src)

        # squares: Pool takes 11 groups (SBUF-only; GPSIMD cannot touch PSUM),
        # ACT 11, DVE 10 — balanced against each engine's PSUM-op load.
        sq_eng = []
        for g in range(RPC // 4):
            if g % 3 == 0:
                sq_eng.append(nc.gpsimd)
            else:
                sq_eng.append(nc.vector if (g % 2) else nc.scalar)
        with tc.tile_pool(name="psA", bufs=2, space="PSUM") as psA, \
             tc.tile_pool(name="psY", bufs=3, space="PSUM") as psY, \
             tc.tile_pool(name="zwork", bufs=3) as zw, \
             tc.tile_pool(name="ypool", bufs=2) as yp:
            # ---- stage C: fp8 DoubleRow pair-bias over own z rows ----
            # DoubleRow requires tile_position (0,0) + out at partition 0,
            # so each row gets its own [32, S] 2-bank PSUM tile; a per-row
            # engine copy packs 4 rows into one [128, S] bf16 tile that
            # leaves via a single bulk DMA per group.
            for grp in range(RPC // 4):
                zq = zw.tile([128, 4, 2, S], F8, tag="zq")
                nc.sync.dma_start(zq[:, :, 0, :], zb[:, ds(4 * grp, 4), :])
                square(sq_eng[grp], zq[:, :, 1, :], zq[:, :, 0, :])
                y4 = yp.tile([128, S], BF16, tag="y4")
                for j in range(4):
                    ps = psY.tile([32, S], FP32, tag="ps")
                    for m in range(4):
                        nc.tensor.matmul(
                            ps[:, ds(256 * m, 256)],
                            lhsT=w01_sb[:],
                            rhs=zq[:, j, :, ds(256 * m, 256)],
                            start=True, stop=True, perf_mode=DR,
                            tile_position=(0, 0))
                    copy(nc.vector if (grp + j) % 2 else nc.scalar,
                         y4[ds(32 * j, 32), :], ps[:])
                nc.sync.dma_start(y_flat[ds(128 * grp, 128)], y4[:])

            # ---- stage B ----
            # kT (padded to HDP per head): [dout_block, t]
            for blk in range(8):
                for ch in range(2):
                    p = psA.tile([128, 512], FP32, tag="pA")
                    for ko in range(6):
                        nc.tensor.matmul(p[:], lhsT=wk_sb[:, ko, ts(blk, 128)],
                                         rhs=sT_sb[:, ko, ts(ch, 512)],
                                         start=(ko == 0), stop=(ko == 5))
                    copy(nc.scalar, kT_sb[:, blk, ts(ch, 512)], p[:])
            # v (padded): [t_block, dout]
            for tb in range(8):
                for ch in range(2):
                    p = psA.tile([128, 512], FP32, tag="pA")
                    for ko in range(6):
                        nc.tensor.matmul(p[:], lhsT=sT_sb[:, ko, ts(tb, 128)],
                                         rhs=wv_sb[:, ko, ts(ch, 512)],
                                         start=(ko == 0), stop=(ko == 5))
                    copy(nc.vector,
                         v_sb[:, tb, ds(8 * ch, 8), 0:HDP],
                         p.rearrange("p (a b) -> p a b", a=8))
            # qT for own rows, scaled by 1/sqrt(hd), bias added
            for blk in range(8):
                p = psA.tile([128, 512], FP32, tag="pA", name="pQ")[:, :RPC]
                for ko in range(6):
                    nc.tensor.matmul(p[:], lhsT=wq_sb[:, ko, ts(blk, 128)],
                                     rhs=sTc_sb[:, ko, :],
                                     start=(ko == 0), stop=(ko == 5))
                nc.scalar.activation(qT_sb[:, blk, :], p[:], AF.Identity,
                                     bias=bq_sb[:, blk:blk + 1], scale=ISQ)
            # g for own rows
            for ch, w in ((0, 512), (1, 256)):
                p = psA.tile([128, 512], FP32, tag="pA")
                for ko in range(6):
                    nc.tensor.matmul(p[:, :w], lhsT=sTc_sb[:, ko, :],
                                     rhs=wg_sb[:, ko, ds(512 * ch, w)],
                                     start=(ko == 0), stop=(ko == 5))
                nc.vector.tensor_copy(g_sb[:, ds(512 * ch, w)], p[:, :w])

        # ---- stage D: r from round-tripped stats (all 64-scaled) ----
        y_rows = y_flat.rearrange("(p a) t -> p a t", a=32)
        nc.sync.dma_start(mu_sb[:], y_rows[:, 16, :])
        nc.sync.dma_start(ez2_sb[:], y_rows[:, 17, :])
        nc.vector.tensor_tensor(var_sb[:], mu_sb[:], mu_sb[:], ALU.mult)
        nc.vector.scalar_tensor_tensor(var_sb[:], ez2_sb[:], WS, var_sb[:],
                                       op0=ALU.mult, op1=ALU.subtract)
        nc.scalar.activation(var_sb[:], var_sb[:], AF.Ln, bias=eps_sb[:])
        nc.scalar.activation(r_sb[:], var_sb[:], AF.Exp, scale=-0.5)

        # ---- stage E: attention per head ----
        with tc.tile_pool(name="psE", bufs=2, space="PSUM") as psE, \
             tc.tile_pool(name="head", bufs=2) as hw_pool:
            for h in range(H):
                po2, blk = 64 * (h % 2), h // 2
                y_h = hw_pool.tile([128, S], BF16, tag="yh")
                nc.sync.dma_start(y_h[:], y_rows[:, h, :])
                t1 = hw_pool.tile([128, S], BF16, tag="t1")
                nc.vector.tensor_tensor(t1[:], y_h[:], r_sb[:], ALU.mult)
                sc = hw_pool.tile([128, S], BF16, tag="sc")
                pq = psE.tile([128, S], FP32, tag="qk")
                for ch in range(2):
                    nc.tensor.matmul(pq[:, ts(ch, 512)],
                                     lhsT=qT_sb[ds(po2, 64), blk, :],
                                     rhs=kT_sb[ds(po2, 64), blk, ts(ch, 512)],
                                     start=True, stop=True)
                nc.vector.tensor_tensor(sc[:], pq[:], t1[:], ALU.add)
                aT = hw_pool.tile([128, 8, 128], BF16, tag="aT")
                for half in range(2):
                    pt = psE.tile([128, 512], BF16, tag="pt")
                    for jj in range(4):
                        nc.tensor.transpose(pt[:, ts(jj, 128)],
                                            sc[:, ts(4 * half + jj, 128)],
                                            ident[:])
                    nc.scalar.activation(aT[:, ds(4 * half, 4), :],
                                         pt.rearrange("p (a b) -> p a b", a=4),
                                         AF.Exp)
                po = psE.tile([128, HDP + 1], FP32, tag="po")
                for tb in range(8):
                    nc.tensor.matmul(po[:], lhsT=aT[:, tb, :],
                                     rhs=v_sb[:, tb, h, :],
                                     start=(tb == 0), stop=(tb == 7))
                dr = hw_pool.tile([128, 1], FP32, tag="dr")
                nc.vector.reciprocal(dr[:], po[:, HDP:HDP + 1])
                nc.vector.tensor_scalar(oall[:, ds(HD * h, HD)], po[:, 0:HD],
                                        dr[:], None, op0=ALU.mult)

            # ---- stage F: gate + output projection ----
            sig = hw_pool.tile([128, D], BF16, tag="sig")
            nc.scalar.activation(sig[:], g_sb[:], AF.Sigmoid)
            og = hw_pool.tile([128, D], BF16, tag="og")
            nc.vector.tensor_tensor(og[:], oall[:], sig[:], ALU.mult)
            ogT = hw_pool.tile([128, 6, 128], BF16, tag="ogT")
            for half, n in ((0, 4), (1, 2)):
                pt = psE.tile([128, 512], BF16, tag="pt")
                for jj in range(n):
                    nc.tensor.transpose(pt[:, ts(jj, 128)],
                                        og[:, ts(4 * half + jj, 128)], ident[:])
                copy(nc.vector if half else nc.scalar,
                     ogT[:, ds(4 * half, n), :],
                     pt.rearrange("p (a b) -> p a b", a=4)[:, 0:n, :])
            out_sb = hw_pool.tile([128, D], FP32, tag="outsb")
            pf = psE.tile([128, S], FP32, tag="qk")
            for ch, w in ((0, 512), (1, 256)):
                for ko in range(6):
                    nc.tensor.matmul(pf[:, ds(512 * ch, w)], lhsT=ogT[:, ko, :],
                                     rhs=wo_sb[:, ko, ds(512 * ch, w)],
                                     start=(ko == 0), stop=(ko == 5))
            copy(nc.vector, out_sb[:], pf[:, :D])
            nc.sync.dma_start(out[:], out_sb[:])

    nc.compile()
    return nc


def _prep(inputs):
    bf = ml_dtypes.bfloat16
    f8 = ml_dtypes.float8_e4m3
    s = np.asarray(inputs["s"], np.float32)[0]
    z = np.asarray(inputs["z"], np.float32)[0]
    Wq = np.asarray(inputs["Wq"], np.float32)
    bq = np.asarray(inputs["bq"], np.float32)
    Wk = np.asarray(inputs["Wk"], np.float32)
    Wv = np.asarray(inputs["Wv"], np.float32)
    Wg = np.asarray(inputs["Wg"], np.float32)
    ln_w = np.asarray(inputs["ln_w"], np.float32)
    ln_b = np.asarray(inputs["ln_b"], np.float32)  # noqa: F841 (softmax-invariant)
    Wz = np.asarray(inputs["Wz"], np.float32)
    Wo = np.asarray(inputs["Wo"], np.float32)

    def pad_rows(W):
        Wp = np.zeros((DP, D), np.float32)
        for h in range(H):
            Wp[h * HDP:h * HDP + HD] = W[h * HD:(h + 1) * HD]
        return Wp

    z8 = z.astype(f8)                            # [S, S, DZ]
    sT = np.ascontiguousarray(s.T).astype(bf)
    WqTp = np.ascontiguousarray(pad_rows(Wq).T).astype(bf)
    WkTp = np.ascontiguousarray(pad_rows(Wk).T).astype(bf)
    WvTp = np.ascontiguousarray(pad_rows(Wv).T).astype(bf)
    WgT = np.ascontiguousarray(Wg.T).astype(bf)
    WoT = np.ascontiguousarray(Wo.T).astype(bf)
    bq_p = np.zeros(DP, np.float32)
    for h in range(H):
        bq_p[h * HDP:h * HDP + HD] = bq[h * HD:(h + 1) * HD]
    bq_p *= ISQ

    Wzp = ln_w[None, :] * Wz                     # [H, DZ]
    c1 = Wzp.sum(-1)                             # [H]
    w01 = np.zeros((DZ, 2, 32), np.float32)
    w01[:, 0, :H] = (Wzp - c1[:, None] / DZ).T   # mean-fold baked in
    w01[:, 0, 16] = 1.0 / DZ                     # 64*mu column (after WS)
    w01[:, 1, 17] = 1.0 / DZ                     # 64*E[z^2] column
    w01 *= WS

    shared = {
        "sT": sT, "WqT": WqTp, "WkT": WkTp, "WvT": WvTp, "WgT": WgT,
        "WoT": WoT, "bqs": bq_p, "w01": w01.astype(f8),
    }
    in_maps = []
    for ci in range(N_CORES):
        rows = slice(ci * RPC, (ci + 1) * RPC)
        m = dict(shared)
        m["zb"] = np.ascontiguousarray(z8[rows].transpose(2, 0, 1))
        m["sTc"] = np.ascontiguousarray(sT[:, rows])
        in_maps.append(m)
    return in_maps


def _install_ntff_hook():
    try:
        import antenv
        from trn_agent_boot.trn_boot import _ntff_profile_via_ctypes
        from concourse import bass_utils
        mod = types.ModuleType("antenv.axon_hooks")
        mod._hook = _ntff_profile_via_ctypes('/opt/axon/libaxon_pjrt.so')
        mod.set_axon_ntff_profile_hook = lambda h: setattr(mod, "_hook", h)
        mod.get_axon_ntff_profile_hook = lambda: mod._hook
        sys.modules["antenv.axon_hooks"] = mod
        antenv.axon_hooks = mod
        bass_utils.upload_artifacts = lambda tmpdir: tmpdir
    except Exception as e:  # profiling is best-effort
        print(f"ntff hook install failed: {e}", file=sys.stderr)


def run(inputs, trace=False):
    from concourse.bass_utils import run_bass_kernel_spmd
    in_maps = _prep(inputs)
    if "nc" not in _CACHE:
        _CACHE["nc"] = _build()
    nc = _CACHE["nc"]
    if trace:
        _install_ntff_hook()
    res = run_bass_kernel_spmd(nc, in_maps, core_ids=list(range(N_CORES)),
                               trace=trace)
    out = np.concatenate([res.results[i]["out"] for i in range(N_CORES)], axis=0)
    return out[None].astype(np.float32), res


def kernel(**inputs) -> np.ndarray:
    out, _ = run(inputs, trace=bool(os.environ.get("KERNEL_TRACE")))
    return out
